# revision 47
# baseline (speedup 1.0000x reference)
# Trainium2 Bass kernel for AoE-style MoE (top-2 of 8 experts).
#
# Two-launch design:
#
#   Launch 1 (data-parallel, 512 tokens/core): the gate. gh = W_A @ x as
#   pair-packed fp32r matmuls (full fp32 operands at bf16 streaming rate),
#   scores = ||gh||_2 per expert via selector matmul + sqrt, then top-2 +
#   softmax with DVE ops. Outputs the dense combine-weight matrix
#   w_all[token, expert] (zero for unselected experts) per core.
#
#   Host: reads w_all, groups tokens by expert (the "all-to-all dispatch by
#   topk_indices" of the sharding spec, done as host data movement), and
#   builds per-expert batches in h-major layout. No arithmetic.
#
#   Launch 2 (expert-parallel, core e = expert e, capacity K2 slots): the
#   FFN. ghg = W_A @ x_g (bf16 recompute), g = W_B^T ghg, up = W_up @ x_g,
#   h = silu(g)*up, y = W_down @ h, then y *= w (combine weight) on-device.
#   Outputs weighted y per slot.
#
#   Host unshard: out[token] = sum of its (at most 2) expert contributions
#   — a pure scatter-add of device-computed partials.
#
# kernel(**inputs) takes full unsharded inputs, returns the full output.

import os
import sys
import types
import numpy as np
import ml_dtypes

E, TOPK, H, F, R = 8, 2, 1024, 2048, 64
B, S = 2, 2048
N = B * S            # 4096 tokens
NCORES = 8
T = N // NCORES      # 512 tokens per core in launch 1

BF16 = ml_dtypes.bfloat16

GATE_DT = os.environ.get("MOE_GATE_DT", "fp32")    # "fp32r" | "fp32"


def _maybe_install_trace_hook():
    if os.environ.get("MOE_TRACE") != "1":
        return False
    try:
        import antenv.axon_hooks  # noqa: F401
        return True
    except ImportError:
        pass
    try:
        if "/root/.axon_site" not in sys.path:
            sys.path.insert(0, "/root/.axon_site")
        from trn_agent_boot.trn_boot import _ntff_profile_via_ctypes
        hook = _ntff_profile_via_ctypes("/opt/axon/libaxon_pjrt.so")
        mod = types.ModuleType("antenv.axon_hooks")
        mod.get_axon_ntff_profile_hook = lambda: hook
        mod.set_axon_ntff_profile_hook = lambda h: None
        sys.modules["antenv.axon_hooks"] = mod
        return True
    except Exception:
        return False


_NC_CACHE = {}
_W_CACHE = {}
LAST_RESULT = None   # small namespace with .exec_time_ns (sum of launches)


class _Result:
    def __init__(self, exec_time_ns, instructions_and_trace, parts):
        self.exec_time_ns = exec_time_ns
        self.instructions_and_trace = instructions_and_trace
        self.parts = parts          # list of per-launch exec times
        self.results = None


# ------------------------------------------------------------------
# Launch 1: gate + top-2 softmax  (data-parallel over tokens)
# ------------------------------------------------------------------
def _build_nc_gate(gate_dt):
    import concourse.mybir as mybir
    import concourse.tile as tile
    from concourse import bacc

    f32 = mybir.dt.float32
    gdt = mybir.dt.float32r if gate_dt == "fp32r" else f32
    AF = mybir.ActivationFunctionType
    OP = mybir.AluOpType
    AX = mybir.AxisListType

    nc = bacc.Bacc("TRN2", target_bir_lowering=False, debug=False,
                   num_devices=NCORES)

    bf16 = mybir.dt.bfloat16
    xT_d = nc.dram_tensor("xT", [128, 8, T], gdt, kind="ExternalInput")
    WApk_d = nc.dram_tensor("WApk", [128, 4, 8, 128], gdt, kind="ExternalInput")
    esel_d = nc.dram_tensor("esel", [128, 4, E], f32, kind="ExternalInput")
    w_d = nc.dram_tensor("w_all", [128, 4, E], f32, kind="ExternalOutput")
    gh_d = nc.dram_tensor("gh_bf", [128, 4, T], bf16, kind="ExternalOutput")

    with tile.TileContext(nc) as tc:
        with tc.tile_pool(name="gp", bufs=1) as gp, \
             tc.tile_pool(name="gps", bufs=2, space="PSUM") as gps, \
             tc.tile_pool(name="gps1", bufs=1, space="PSUM") as gps1:
            # DMA order: wapk pr0 (small, feeds prewarm), xT chunks, then
            # the remaining wapk pr-rounds (consumed in pr order).
            wapk_pr = []
            w0 = gp.tile([128, 8, 128], gdt, tag="wapk0", name="wapk0")
            nc.sync.dma_start(w0[:, 0:2, :], WApk_d[:, 0, 0:2, :])
            wapk_pr.append(w0)
            xT_sb = gp.tile([128, 8, T], gdt, tag="xT_sb")
            nc.sync.dma_start(xT_sb[:, 0, :], xT_d[:, 0, :])
            nc.sync.dma_start(w0[:, 2:8, :], WApk_d[:, 0, 2:8, :])
            for k in range(1, 8):
                nc.sync.dma_start(xT_sb[:, k, :], xT_d[:, k, :])
            xTf_k = [xT_sb[:, k, :] for k in range(8)]
            esel = gp.tile([128, 4, E], f32, tag="esel")
            nc.sync.dma_start(esel[:], esel_d[:])
            for pr in range(1, 4):
                wk = gp.tile([128, 8, 128], gdt, tag=f"wapk{pr}",
                             name=f"wapk{pr}")
                nc.sync.dma_start(wk[:], WApk_d[:, pr, :, :])
                wapk_pr.append(wk)

            # preload the sqrt activation table early (esel is non-negative)
            sqwarm = gp.tile([128, 4, E], f32, tag="sqwarm")
            nc.scalar.sqrt(sqwarm[:], esel[:])

            gh2 = gp.tile([128, 4, T], f32, tag="gh2")
            ghbf = gp.tile([128, 4, T], bf16, tag="ghbf")
            s_all = gp.tile([128, 4, E], f32, tag="s_all")
            stp = [gps1.tile([128, E], f32, tag=f"stok{c}", name=f"stok{c}")
                   for c in range(4)]

            def scores_for(pr):
                for c in range(4):
                    nc.tensor.matmul(stp[c][:],
                                     gh2[:, pr, c * 128:(c + 1) * 128],
                                     esel[:, pr, :],
                                     start=(pr == 0), stop=(pr == 3))

            # score matmuls for round pr issue after gate round pr+1 so the
            # PE never waits on the scalar-engine square
            for pr in range(4):
                ghp = gps.tile([128, T], f32, tag="gh")
                for k in range(8):
                    nc.tensor.matmul(ghp[:], wapk_pr[pr][:, k, :],
                                     xTf_k[k],
                                     start=(k == 0), stop=(k == 7))
                nc.scalar.square(gh2[:, pr, :], ghp[:])
                nc.vector.tensor_copy(ghbf[:, pr, :], ghp[:])
                if pr >= 1:
                    scores_for(pr - 1)
            scores_for(3)
            # ship fp32-exact gh (as bf16) for launch 2's g-projection;
            # the DMA overlaps the softmax chain below
            nc.sync.dma_start(gh_d[:], ghbf[:])
            for c in range(4):
                nc.scalar.sqrt(s_all[:, c, :], stp[c][:])

            # top-2 + softmax over E per token
            m1 = gp.tile([128, 4], f32, tag="m1")
            nc.vector.reduce_max(m1[:], s_all[:], axis=AX.X)
            m1b = m1[:, :, None].to_broadcast((128, 4, E))
            eqm = gp.tile([128, 4, E], f32, tag="eqm")
            nc.vector.tensor_tensor(eqm[:], s_all[:], m1b, OP.is_ge)
            s2 = gp.tile([128, 4, E], f32, tag="s2")
            nc.vector.scalar_tensor_tensor(s2[:], eqm[:], -1e30, s_all[:],
                                           OP.mult, OP.add)
            m2 = gp.tile([128, 4], f32, tag="m2")
            nc.vector.reduce_max(m2[:], s2[:], axis=AX.X)
            m2b = m2[:, :, None].to_broadcast((128, 4, E))

            d1 = gp.tile([128, 4, E], f32, tag="d1")
            nc.vector.tensor_tensor(d1[:], s_all[:], m1b, OP.subtract)
            e1 = gp.tile([128, 4, E], f32, tag="e1")
            nc.scalar.activation(e1[:], d1[:], AF.Exp)
            dm2 = gp.tile([128, 4], f32, tag="dm2")
            nc.vector.tensor_tensor(dm2[:], m2[:], m1[:], OP.subtract)
            em2 = gp.tile([128, 4], f32, tag="em2")
            nc.scalar.activation(em2[:], dm2[:], AF.Exp)
            den = gp.tile([128, 4], f32, tag="den")
            nc.vector.tensor_scalar_add(den[:], em2[:], 1.0)
            rec = gp.tile([128, 4], f32, tag="rec")
            nc.vector.reciprocal(rec[:], den[:])
            recb = rec[:, :, None].to_broadcast((128, 4, E))
            mask2 = gp.tile([128, 4, E], f32, tag="mask2")
            nc.vector.tensor_tensor(mask2[:], s_all[:], m2b, OP.is_ge)
            wm = gp.tile([128, 4, E], f32, tag="wm")
            nc.vector.tensor_tensor(wm[:], e1[:], mask2[:], OP.mult)
            w_all = gp.tile([128, 4, E], f32, tag="w_all")
            nc.vector.tensor_tensor(w_all[:], wm[:], recb, OP.mult)
            nc.sync.dma_start(w_d[:], w_all[:])

    nc.compile()
    return nc


# ------------------------------------------------------------------
# Launch 2: expert FFN over K2 gathered slots (expert-parallel)
# ------------------------------------------------------------------
def _build_nc_ffn(K2):
    import concourse.mybir as mybir
    import concourse.tile as tile
    from concourse import bacc

    f32 = mybir.dt.float32
    bf16 = mybir.dt.bfloat16
    AF = mybir.ActivationFunctionType
    OP = mybir.AluOpType

    # slot chunks (psum moving-dim limit is 512); balanced widths so no
    # chunk is so narrow that LDWEIGHTS dominates the streaming time
    nch = (K2 + 511) // 512
    base = K2 // nch
    SC = []
    s0 = 0
    for i in range(nch):
        sw = base + (1 if i < K2 - base * nch else 0)
        SC.append((s0, sw))
        s0 += sw
    assert s0 == K2

    nc = bacc.Bacc("TRN2", target_bir_lowering=False, debug=False,
                   num_devices=NCORES)

    XG_d = nc.dram_tensor("XG", [128, 8, K2], bf16, kind="ExternalInput")
    GHG_d = nc.dram_tensor("GHG", [128, K2], bf16, kind="ExternalInput")
    WB_d = nc.dram_tensor("WB", [128, F], bf16, kind="ExternalInput")
    # ft-major W_up: [h%128, ft, h//128, f%128] so the first f-tile's
    # weights land right after XG instead of at the end of the stream
    WUP_d = nc.dram_tensor("WUP", [128, 16, 8, 128], bf16,
                           kind="ExternalInput")
    WDN_d = nc.dram_tensor("WDN", [128, 16, H], bf16, kind="ExternalInput")
    WBC_d = nc.dram_tensor("WBC", [128, K2], f32, kind="ExternalInput")
    Y_d = nc.dram_tensor("Y", [128, 8, K2], bf16, kind="ExternalOutput")

    with tile.TileContext(nc) as tc:
        with tc.tile_pool(name="pp", bufs=1) as pp, \
             tc.tile_pool(name="sil_p", bufs=4) as sil_p, \
             tc.tile_pool(name="y_p", bufs=4) as y_p:
            # DMA issue order = need order: GHG (prewarm + g), XG per-k,
            # WB, WUP ft-pairs, WBC, WDN (pass 2 only) last.
            ghg = pp.tile([128, K2], bf16, tag="ghg")
            nc.sync.dma_start(ghg[:], GHG_d[:])
            wb = pp.tile([128, F], bf16, tag="wb")
            nc.sync.dma_start(wb[:], WB_d[:])
            xg = pp.tile([128, 8, K2], bf16, tag="xg")
            for k in range(8):
                nc.sync.dma_start(xg[:, k, :], XG_d[:, k, :])
            wup = pp.tile([128, 16, 8, 128], bf16, tag="wup")
            for fp2 in range(8):
                nc.sync.dma_start(wup[:, fp2 * 2:(fp2 + 1) * 2, :, :],
                                  WUP_d[:, fp2 * 2:(fp2 + 1) * 2, :, :])
            wbc = pp.tile([128, K2], f32, tag="wbc")
            nc.sync.dma_start(wbc[:], WBC_d[:])
            wdn = pp.tile([128, 16, H], bf16, tag="wdn")
            for fc in range(4):
                nc.sync.dma_start(wdn[:, fc * 4:(fc + 1) * 4, :],
                                  WDN_d[:, fc * 4:(fc + 1) * 4, :])

            h_sb = pp.tile([128, 16, K2], bf16, tag="h_sb")

            with tc.tile_pool(name="ps_w", bufs=1, space="PSUM") as ps_w:
                # prewarm the PE from t~4us: memset needs no DMA, so these
                # junk matmuls warm HAM long before the first input lands
                jsrc = pp.tile([128, 512], bf16, tag="jsrc")
                nc.vector.memset(jsrc[:], 1.0)
                junk = ps_w.tile([64, 512], f32, tag="junk")
                for j in range(22):
                    nc.tensor.matmul(junk[:], jsrc[:, 0:64], jsrc[:],
                                     start=True, stop=True)

            # pass 1: h = silu(W_B^T ghg) * (W_up @ x_g), f-major.
            # The g+silu for the first NH f-tiles is hoisted ahead: it only
            # needs GHG+WB (land early), filling the PE-idle window while
            # XG/WUP stream in. NH is kept small so the scalar-engine silu
            # drain stays off the critical path (hoisting all 16 tiles made
            # pass 1 scalar-bound).
            NH = 4
            sil_sb = pp.tile([128, NH, K2], bf16, tag="sil_sb")
            with tc.tile_pool(name="ps_g", bufs=4, space="PSUM") as ps_g, \
                 tc.tile_pool(name="ps_up", bufs=3, space="PSUM") as ps_up:
                for fg in range(NH):
                    for (s0, sw) in SC:
                        gps = ps_g.tile([128, 512], f32, tag="g")
                        nc.tensor.matmul(gps[:, :sw],
                                         wb[:, fg * 128:(fg + 1) * 128],
                                         ghg[:, s0:s0 + sw],
                                         start=True, stop=True)
                        nc.scalar.activation(sil_sb[:, fg, s0:s0 + sw],
                                             gps[:, :sw], AF.Silu)
                for fg in range(16):
                    for (s0, sw) in SC:
                        if fg >= NH:
                            gps = ps_g.tile([128, 512], f32, tag="g")
                            nc.tensor.matmul(gps[:, :sw],
                                             wb[:, fg * 128:(fg + 1) * 128],
                                             ghg[:, s0:s0 + sw],
                                             start=True, stop=True)
                        ups = ps_up.tile([128, 512], f32, tag="up")
                        for k in range(8):
                            nc.tensor.matmul(
                                ups[:, :sw],
                                wup[:, fg, k, :],
                                xg[:, k, s0:s0 + sw],
                                start=(k == 0), stop=(k == 7))
                        if fg >= NH:
                            sil = sil_p.tile([128, 512], bf16, tag="sil")
                            nc.scalar.activation(sil[:, :sw], gps[:, :sw],
                                                 AF.Silu)
                            nc.vector.tensor_tensor(h_sb[:, fg, s0:s0 + sw],
                                                    sil[:, :sw], ups[:, :sw],
                                                    OP.mult)
                        else:
                            nc.vector.tensor_tensor(h_sb[:, fg, s0:s0 + sw],
                                                    sil_sb[:, fg, s0:s0 + sw],
                                                    ups[:, :sw], OP.mult)

            # pass 2: y = (W_down @ h) * w  (h-major out, weighted on device)
            with tc.tile_pool(name="ps_y", bufs=3, space="PSUM") as ps_y:
                for hh in range(8):
                    for (s0, sw) in SC:
                        yps = ps_y.tile([128, 512], f32, tag="y")
                        for fg in range(16):
                            nc.tensor.matmul(
                                yps[:, :sw],
                                wdn[:, fg, hh * 128:(hh + 1) * 128],
                                h_sb[:, fg, s0:s0 + sw],
                                start=(fg == 0), stop=(fg == 15))
                        ysb = y_p.tile([128, 512], bf16, tag="ysb")
                        nc.vector.tensor_tensor(ysb[:, :sw], yps[:, :sw],
                                                wbc[:, s0:s0 + sw], OP.mult)
                        nc.sync.dma_start(Y_d[:, hh, s0:s0 + sw], ysb[:, :sw])

    nc.compile()
    return nc


def _get_nc(key, builder, *args):
    if key not in _NC_CACHE:
        _NC_CACHE[key] = builder(*args)
    return _NC_CACHE[key]


# ------------------------------------------------------------------
# Host-side input prep
# ------------------------------------------------------------------
def _prep_gate_inputs(x2d, W_A):
    f32 = np.float32
    xT = np.ascontiguousarray(x2d.T)                        # [H, N]
    xT_arr = np.ascontiguousarray(
        xT.reshape(8, 128, N).transpose(1, 0, 2))           # [128, 8, N]

    WA_t = W_A.transpose(0, 2, 1).reshape(E, 8, 128, R)     # [E, k, p, R]
    WApk = np.zeros((128, 4, 8, 128), dtype=f32)
    for pr in range(4):
        WApk[:, pr, :, 0:64] = WA_t[2 * pr].transpose(1, 0, 2)
        WApk[:, pr, :, 64:128] = WA_t[2 * pr + 1].transpose(1, 0, 2)

    esel = np.zeros((128, 4, E), dtype=f32)
    for pr in range(4):
        esel[0:64, pr, 2 * pr] = 1.0
        esel[64:128, pr, 2 * pr + 1] = 1.0

    in_maps = []
    for c in range(NCORES):
        in_maps.append(dict(
            xT=np.ascontiguousarray(xT_arr[:, :, c * T:(c + 1) * T]),
            WApk=WApk, esel=esel))
    return in_maps


def _prep_expert_weights(W_A, W_B, W_up, W_down):
    key = "w"
    if key in _W_CACHE:
        return _W_CACHE[key]
    # WB: [E,F,R] -> [E, 128(R pad), F]
    WBh = np.zeros((E, 128, F), dtype=BF16)
    WBh[:, :R, :] = W_B.transpose(0, 2, 1).astype(BF16)
    # WUP: [E,F,H] -> ft-major [E, 128, 16, 8, 128]
    #   [e, h%128, ft, h//128, f%128], f = ft*128 + (f%128)
    WUPh = np.ascontiguousarray(
        W_up.transpose(0, 2, 1).reshape(E, 8, 128, 16, 128)
        .transpose(0, 2, 3, 1, 4)).astype(BF16)
    # WDN: [E,H,F] -> [E, 128, 16, H]  (f = k*128 + p)
    WDNh = np.ascontiguousarray(
        W_down.transpose(0, 2, 1).reshape(E, 16, 128, H).transpose(0, 2, 1, 3)
    ).astype(BF16)
    _W_CACHE[key] = (WBh, WUPh, WDNh)
    return _W_CACHE[key]


def kernel(hidden_states, W_A, W_B, W_up, W_down):
    global LAST_RESULT
    trace = _maybe_install_trace_hook()
    from concourse import bass_utils

    f32 = np.float32
    x2d = np.ascontiguousarray(
        np.asarray(hidden_states, dtype=f32).reshape(N, H))
    W_A = np.asarray(W_A, dtype=f32)
    W_B = np.asarray(W_B, dtype=f32)
    W_up = np.asarray(W_up, dtype=f32)
    W_down = np.asarray(W_down, dtype=f32)

    # ---- launch 1: gate ----
    nc1 = _get_nc("gate_" + GATE_DT, _build_nc_gate, GATE_DT)
    in1 = _prep_gate_inputs(x2d, W_A)
    res1 = bass_utils.run_bass_kernel_spmd(
        nc1, in1, core_ids=list(range(NCORES)), trace=trace)

    # w_full[token, expert] (device-computed dense combine weights) and
    # gh (device fp32 gate activations, shipped as bf16) per expert
    w_full = np.empty((N, E), dtype=f32)
    ghG = np.empty((E, R, N), dtype=BF16)
    for c in range(NCORES):
        arr = res1.results[c]["w_all"]                      # [128, 4, E]
        w_full[c * T:(c + 1) * T] = arr.transpose(1, 0, 2).reshape(T, E)
        ghc = res1.results[c]["gh_bf"]                      # [128, 4, T]
        for pr in range(4):
            for half in range(2):
                ghG[2 * pr + half, :, c * T:(c + 1) * T] = \
                    ghc[half * 64:(half + 1) * 64, pr, :]

    # ---- host: group tokens by expert (all-to-all dispatch) ----
    tok_lists = []
    counts = []
    for e in range(E):
        tl = np.nonzero(w_full[:, e] > 0.0)[0]
        tok_lists.append(tl)
        counts.append(len(tl))
    maxc = max(counts)
    K2 = ((maxc + 23 + 127) // 128) * 128                   # capacity w/ pad

    WBh, WUPh, WDNh = _prep_expert_weights(W_A, W_B, W_up, W_down)
    x_bf = x2d.astype(BF16)

    nc2 = _get_nc(("ffn", K2), _build_nc_ffn, K2)
    in2 = []
    for e in range(E):
        tl = tok_lists[e]
        cnt = counts[e]
        xg = np.zeros((H, K2), dtype=BF16)
        xg[:, :cnt] = x_bf[tl].T                            # [H, cnt]
        XG = np.ascontiguousarray(
            xg.reshape(8, 128, K2).transpose(1, 0, 2))      # [128, 8, K2]
        GHG = np.zeros((128, K2), dtype=BF16)
        GHG[:R, :cnt] = ghG[e][:, tl]
        wbc = np.zeros((128, K2), dtype=f32)
        wbc[:, :cnt] = w_full[tl, e][None, :]
        in2.append(dict(XG=XG, GHG=GHG, WB=WBh[e], WUP=WUPh[e],
                        WDN=WDNh[e], WBC=wbc))
    res2 = bass_utils.run_bass_kernel_spmd(
        nc2, in2, core_ids=list(range(NCORES)), trace=trace)

    # ---- host unshard: scatter-add the weighted expert partials ----
    out = np.zeros((N, H), dtype=f32)
    for e in range(E):
        cnt = counts[e]
        Y = res2.results[e]["Y"]                            # [128, 8, K2] bf16
        y = Y.transpose(2, 1, 0).reshape(K2, H)[:cnt].astype(f32)
        out[tok_lists[e]] += y

    t1 = res1.exec_time_ns
    t2 = res2.exec_time_ns
    total = (t1 or 0) + (t2 or 0) if (t1 is not None or t2 is not None) else None
    tr = res2.instructions_and_trace or res1.instructions_and_trace
    LAST_RESULT = _Result(total if (t1 or t2) else None, tr, [t1, t2])

    return out.reshape(B, S, H)


# revision 48
# speedup vs baseline: 1.1857x; 1.1857x over previous
# Trainium2 Bass kernel for AoE-style MoE (top-2 of 8 experts).
#
# Two-launch design:
#
#   Launch 1 (data-parallel, 512 tokens/core): the gate. gh = W_A @ x as
#   pair-packed fp32r matmuls (full fp32 operands at bf16 streaming rate),
#   scores = ||gh||_2 per expert via selector matmul + sqrt, then top-2 +
#   softmax with DVE ops. Outputs the dense combine-weight matrix
#   w_all[token, expert] (zero for unselected experts) per core.
#
#   Host: reads w_all, groups tokens by expert (the "all-to-all dispatch by
#   topk_indices" of the sharding spec, done as host data movement), and
#   builds per-expert batches in h-major layout. No arithmetic.
#
#   Launch 2 (expert-parallel, core e = expert e, capacity K2 slots): the
#   FFN. ghg = W_A @ x_g (bf16 recompute), g = W_B^T ghg, up = W_up @ x_g,
#   h = silu(g)*up, y = W_down @ h, then y *= w (combine weight) on-device.
#   Outputs weighted y per slot.
#
#   Host unshard: out[token] = sum of its (at most 2) expert contributions
#   — a pure scatter-add of device-computed partials.
#
# kernel(**inputs) takes full unsharded inputs, returns the full output.

import os
import sys
import types
import numpy as np
import ml_dtypes

E, TOPK, H, F, R = 8, 2, 1024, 2048, 64
B, S = 2, 2048
N = B * S            # 4096 tokens
NCORES = 8
T = N // NCORES      # 512 tokens per core in launch 1

BF16 = ml_dtypes.bfloat16

GATE_DT = os.environ.get("MOE_GATE_DT", "fp32")    # "fp32r" | "fp32"


def _maybe_install_trace_hook():
    if os.environ.get("MOE_TRACE") != "1":
        return False
    try:
        import antenv.axon_hooks  # noqa: F401
        return True
    except ImportError:
        pass
    try:
        if "/root/.axon_site" not in sys.path:
            sys.path.insert(0, "/root/.axon_site")
        from trn_agent_boot.trn_boot import _ntff_profile_via_ctypes
        hook = _ntff_profile_via_ctypes("/opt/axon/libaxon_pjrt.so")
        mod = types.ModuleType("antenv.axon_hooks")
        mod.get_axon_ntff_profile_hook = lambda: hook
        mod.set_axon_ntff_profile_hook = lambda h: None
        sys.modules["antenv.axon_hooks"] = mod
        return True
    except Exception:
        return False


_NC_CACHE = {}
_W_CACHE = {}
LAST_RESULT = None   # small namespace with .exec_time_ns (sum of launches)


class _Result:
    def __init__(self, exec_time_ns, instructions_and_trace, parts):
        self.exec_time_ns = exec_time_ns
        self.instructions_and_trace = instructions_and_trace
        self.parts = parts          # list of per-launch exec times
        self.results = None


# ------------------------------------------------------------------
# Launch 1: gate + top-2 softmax  (data-parallel over tokens)
# ------------------------------------------------------------------
def _build_nc_gate(gate_dt):
    import concourse.mybir as mybir
    import concourse.tile as tile
    from concourse import bacc

    f32 = mybir.dt.float32
    gdt = mybir.dt.float32r if gate_dt == "fp32r" else f32
    AF = mybir.ActivationFunctionType
    OP = mybir.AluOpType
    AX = mybir.AxisListType

    nc = bacc.Bacc("TRN2", target_bir_lowering=False, debug=False,
                   num_devices=NCORES)

    bf16 = mybir.dt.bfloat16
    xT_d = nc.dram_tensor("xT", [128, 8, T], gdt, kind="ExternalInput")
    WApk_d = nc.dram_tensor("WApk", [128, 4, 8, 128], gdt, kind="ExternalInput")
    esel_d = nc.dram_tensor("esel", [128, 4, E], f32, kind="ExternalInput")
    w_d = nc.dram_tensor("w_all", [128, 4, E], f32, kind="ExternalOutput")
    gh_d = nc.dram_tensor("gh_bf", [128, 4, T], bf16, kind="ExternalOutput")

    with tile.TileContext(nc) as tc:
        with tc.tile_pool(name="gp", bufs=1) as gp, \
             tc.tile_pool(name="gps", bufs=2, space="PSUM") as gps, \
             tc.tile_pool(name="gps1", bufs=1, space="PSUM") as gps1:
            # DMA order: wapk pr0 (small, feeds prewarm), xT chunks, then
            # the remaining wapk pr-rounds (consumed in pr order).
            wapk_pr = []
            w0 = gp.tile([128, 8, 128], gdt, tag="wapk0", name="wapk0")
            nc.sync.dma_start(w0[:, 0:4, :], WApk_d[:, 0, 0:4, :])
            wapk_pr.append(w0)
            xT_sb = gp.tile([128, 8, T], gdt, tag="xT_sb")
            nc.sync.dma_start(xT_sb[:, 0, :], xT_d[:, 0, :])
            nc.sync.dma_start(w0[:, 4:8, :], WApk_d[:, 0, 4:8, :])
            for k in range(1, 8):
                nc.sync.dma_start(xT_sb[:, k, :], xT_d[:, k, :])
            xTf_k = [xT_sb[:, k, :] for k in range(8)]
            esel = gp.tile([128, 4, E], f32, tag="esel")
            nc.sync.dma_start(esel[:], esel_d[:])
            for pr in range(1, 4):
                wk = gp.tile([128, 8, 128], gdt, tag=f"wapk{pr}",
                             name=f"wapk{pr}")
                nc.sync.dma_start(wk[:], WApk_d[:, pr, :, :])
                wapk_pr.append(wk)

            # preload the sqrt activation table early (esel is non-negative)
            sqwarm = gp.tile([128, 4, E], f32, tag="sqwarm")
            nc.scalar.sqrt(sqwarm[:], esel[:])

            gh2 = gp.tile([128, 4, T], f32, tag="gh2")
            ghbf = gp.tile([128, 4, T], bf16, tag="ghbf")
            s_all = gp.tile([128, 4, E], f32, tag="s_all")
            stp = [gps1.tile([128, E], f32, tag=f"stok{c}", name=f"stok{c}")
                   for c in range(4)]

            def scores_for(pr):
                for c in range(4):
                    nc.tensor.matmul(stp[c][:],
                                     gh2[:, pr, c * 128:(c + 1) * 128],
                                     esel[:, pr, :],
                                     start=(pr == 0), stop=(pr == 3))

            # score matmuls for round pr issue after gate round pr+1 so the
            # PE never waits on the scalar-engine square
            for pr in range(4):
                ghp = gps.tile([128, T], f32, tag="gh")
                for k in range(8):
                    nc.tensor.matmul(ghp[:], wapk_pr[pr][:, k, :],
                                     xTf_k[k],
                                     start=(k == 0), stop=(k == 7))
                nc.scalar.square(gh2[:, pr, :], ghp[:])
                nc.vector.tensor_copy(ghbf[:, pr, :], ghp[:])
                if pr >= 1:
                    scores_for(pr - 1)
            scores_for(3)
            # ship fp32-exact gh (as bf16) for launch 2's g-projection;
            # the DMA overlaps the softmax chain below
            nc.sync.dma_start(gh_d[:], ghbf[:])
            for c in range(4):
                nc.scalar.sqrt(s_all[:, c, :], stp[c][:])

            # top-2 + softmax over E per token
            m1 = gp.tile([128, 4], f32, tag="m1")
            nc.vector.reduce_max(m1[:], s_all[:], axis=AX.X)
            m1b = m1[:, :, None].to_broadcast((128, 4, E))
            eqm = gp.tile([128, 4, E], f32, tag="eqm")
            nc.vector.tensor_tensor(eqm[:], s_all[:], m1b, OP.is_ge)
            s2 = gp.tile([128, 4, E], f32, tag="s2")
            nc.vector.scalar_tensor_tensor(s2[:], eqm[:], -1e30, s_all[:],
                                           OP.mult, OP.add)
            m2 = gp.tile([128, 4], f32, tag="m2")
            nc.vector.reduce_max(m2[:], s2[:], axis=AX.X)
            m2b = m2[:, :, None].to_broadcast((128, 4, E))

            d1 = gp.tile([128, 4, E], f32, tag="d1")
            nc.vector.tensor_tensor(d1[:], s_all[:], m1b, OP.subtract)
            e1 = gp.tile([128, 4, E], f32, tag="e1")
            nc.scalar.activation(e1[:], d1[:], AF.Exp)
            dm2 = gp.tile([128, 4], f32, tag="dm2")
            nc.vector.tensor_tensor(dm2[:], m2[:], m1[:], OP.subtract)
            em2 = gp.tile([128, 4], f32, tag="em2")
            nc.scalar.activation(em2[:], dm2[:], AF.Exp)
            den = gp.tile([128, 4], f32, tag="den")
            nc.vector.tensor_scalar_add(den[:], em2[:], 1.0)
            rec = gp.tile([128, 4], f32, tag="rec")
            nc.vector.reciprocal(rec[:], den[:])
            recb = rec[:, :, None].to_broadcast((128, 4, E))
            mask2 = gp.tile([128, 4, E], f32, tag="mask2")
            nc.vector.tensor_tensor(mask2[:], s_all[:], m2b, OP.is_ge)
            wm = gp.tile([128, 4, E], f32, tag="wm")
            nc.vector.tensor_tensor(wm[:], e1[:], mask2[:], OP.mult)
            w_all = gp.tile([128, 4, E], f32, tag="w_all")
            nc.vector.tensor_tensor(w_all[:], wm[:], recb, OP.mult)
            nc.sync.dma_start(w_d[:], w_all[:])

    nc.compile()
    return nc


# ------------------------------------------------------------------
# Launch 2: expert FFN over K2 gathered slots (expert-parallel)
# ------------------------------------------------------------------
def _build_nc_ffn(K2):
    import concourse.mybir as mybir
    import concourse.tile as tile
    from concourse import bacc

    f32 = mybir.dt.float32
    bf16 = mybir.dt.bfloat16
    AF = mybir.ActivationFunctionType
    OP = mybir.AluOpType

    # slot chunks (psum moving-dim limit is 512); balanced widths so no
    # chunk is so narrow that LDWEIGHTS dominates the streaming time
    nch = (K2 + 511) // 512
    base = K2 // nch
    SC = []
    s0 = 0
    for i in range(nch):
        sw = base + (1 if i < K2 - base * nch else 0)
        SC.append((s0, sw))
        s0 += sw
    assert s0 == K2

    nc = bacc.Bacc("TRN2", target_bir_lowering=False, debug=False,
                   num_devices=NCORES)

    XG_d = nc.dram_tensor("XG", [128, 8, K2], bf16, kind="ExternalInput")
    GHG_d = nc.dram_tensor("GHG", [128, K2], bf16, kind="ExternalInput")
    WB_d = nc.dram_tensor("WB", [128, F], bf16, kind="ExternalInput")
    # ft-major W_up: [h%128, ft, h//128, f%128] so the first f-tile's
    # weights land right after XG instead of at the end of the stream
    WUP_d = nc.dram_tensor("WUP", [128, 16, 8, 128], bf16,
                           kind="ExternalInput")
    WDN_d = nc.dram_tensor("WDN", [128, 16, H], bf16, kind="ExternalInput")
    WBC_d = nc.dram_tensor("WBC", [128, K2], f32, kind="ExternalInput")
    Y_d = nc.dram_tensor("Y", [128, 8, K2], bf16, kind="ExternalOutput")

    with tile.TileContext(nc) as tc:
        with tc.tile_pool(name="pp", bufs=1) as pp, \
             tc.tile_pool(name="sil_p", bufs=4) as sil_p, \
             tc.tile_pool(name="y_p", bufs=4) as y_p:
            # DMA issue order = need order: GHG (prewarm + g), XG per-k,
            # WB, WUP ft-pairs, WBC, WDN (pass 2 only) last.
            ghg = pp.tile([128, K2], bf16, tag="ghg")
            nc.sync.dma_start(ghg[:], GHG_d[:])
            wb = pp.tile([128, F], bf16, tag="wb")
            nc.sync.dma_start(wb[:], WB_d[:])
            xg = pp.tile([128, 8, K2], bf16, tag="xg")
            for k in range(8):
                nc.sync.dma_start(xg[:, k, :], XG_d[:, k, :])
            wup = pp.tile([128, 16, 8, 128], bf16, tag="wup")
            for fp2 in range(8):
                nc.sync.dma_start(wup[:, fp2 * 2:(fp2 + 1) * 2, :, :],
                                  WUP_d[:, fp2 * 2:(fp2 + 1) * 2, :, :])
            wbc = pp.tile([128, K2], f32, tag="wbc")
            nc.sync.dma_start(wbc[:], WBC_d[:])
            wdn = pp.tile([128, 16, H], bf16, tag="wdn")
            for fc in range(4):
                nc.sync.dma_start(wdn[:, fc * 4:(fc + 1) * 4, :],
                                  WDN_d[:, fc * 4:(fc + 1) * 4, :])

            h_sb = pp.tile([128, 16, K2], bf16, tag="h_sb")

            with tc.tile_pool(name="ps_w", bufs=1, space="PSUM") as ps_w:
                # prewarm the PE from t~4us: memset needs no DMA, so these
                # junk matmuls warm HAM long before the first input lands
                jsrc = pp.tile([128, 512], bf16, tag="jsrc")
                nc.vector.memset(jsrc[:], 1.0)
                junk = ps_w.tile([64, 512], f32, tag="junk")
                for j in range(26):
                    nc.tensor.matmul(junk[:], jsrc[:, 0:64], jsrc[:],
                                     start=True, stop=True)

            # pass 1: h = silu(W_B^T ghg) * (W_up @ x_g), f-major.
            # The g+silu for the first NH f-tiles is hoisted ahead: it only
            # needs GHG+WB (land early), filling the PE-idle window while
            # XG/WUP stream in. NH is kept small so the scalar-engine silu
            # drain stays off the critical path (hoisting all 16 tiles made
            # pass 1 scalar-bound).
            NH = 4
            sil_sb = pp.tile([128, NH, K2], bf16, tag="sil_sb")
            with tc.tile_pool(name="ps_g", bufs=4, space="PSUM") as ps_g, \
                 tc.tile_pool(name="ps_up", bufs=2, space="PSUM") as ps_up:
                for fg in range(NH):
                    for (s0, sw) in SC:
                        gps = ps_g.tile([128, 512], f32, tag="g")
                        nc.tensor.matmul(gps[:, :sw],
                                         wb[:, fg * 128:(fg + 1) * 128],
                                         ghg[:, s0:s0 + sw],
                                         start=True, stop=True)
                        nc.scalar.activation(sil_sb[:, fg, s0:s0 + sw],
                                             gps[:, :sw], AF.Silu)
                for fg in range(16):
                    for (s0, sw) in SC:
                        if fg >= NH:
                            gps = ps_g.tile([128, 512], f32, tag="g")
                            nc.tensor.matmul(gps[:, :sw],
                                             wb[:, fg * 128:(fg + 1) * 128],
                                             ghg[:, s0:s0 + sw],
                                             start=True, stop=True)
                        ups = ps_up.tile([128, 512], f32, tag="up")
                        for k in range(8):
                            nc.tensor.matmul(
                                ups[:, :sw],
                                wup[:, fg, k, :],
                                xg[:, k, s0:s0 + sw],
                                start=(k == 0), stop=(k == 7))
                        if fg >= NH:
                            sil = sil_p.tile([128, 512], bf16, tag="sil")
                            nc.scalar.activation(sil[:, :sw], gps[:, :sw],
                                                 AF.Silu)
                            nc.vector.tensor_tensor(h_sb[:, fg, s0:s0 + sw],
                                                    sil[:, :sw], ups[:, :sw],
                                                    OP.mult)
                        else:
                            nc.vector.tensor_tensor(h_sb[:, fg, s0:s0 + sw],
                                                    sil_sb[:, fg, s0:s0 + sw],
                                                    ups[:, :sw], OP.mult)

            # pass 2: y = (W_down @ h) * w  (h-major out, weighted on device)
            with tc.tile_pool(name="ps_y", bufs=2, space="PSUM") as ps_y:
                for hh in range(8):
                    for (s0, sw) in SC:
                        yps = ps_y.tile([128, 512], f32, tag="y")
                        for fg in range(16):
                            nc.tensor.matmul(
                                yps[:, :sw],
                                wdn[:, fg, hh * 128:(hh + 1) * 128],
                                h_sb[:, fg, s0:s0 + sw],
                                start=(fg == 0), stop=(fg == 15))
                        ysb = y_p.tile([128, 512], bf16, tag="ysb")
                        nc.vector.tensor_tensor(ysb[:, :sw], yps[:, :sw],
                                                wbc[:, s0:s0 + sw], OP.mult)
                        nc.sync.dma_start(Y_d[:, hh, s0:s0 + sw], ysb[:, :sw])

    nc.compile()
    return nc


def _get_nc(key, builder, *args):
    if key not in _NC_CACHE:
        _NC_CACHE[key] = builder(*args)
    return _NC_CACHE[key]


# ------------------------------------------------------------------
# Host-side input prep
# ------------------------------------------------------------------
def _prep_gate_inputs(x2d, W_A):
    f32 = np.float32
    xT = np.ascontiguousarray(x2d.T)                        # [H, N]
    xT_arr = np.ascontiguousarray(
        xT.reshape(8, 128, N).transpose(1, 0, 2))           # [128, 8, N]

    WA_t = W_A.transpose(0, 2, 1).reshape(E, 8, 128, R)     # [E, k, p, R]
    WApk = np.zeros((128, 4, 8, 128), dtype=f32)
    for pr in range(4):
        WApk[:, pr, :, 0:64] = WA_t[2 * pr].transpose(1, 0, 2)
        WApk[:, pr, :, 64:128] = WA_t[2 * pr + 1].transpose(1, 0, 2)

    esel = np.zeros((128, 4, E), dtype=f32)
    for pr in range(4):
        esel[0:64, pr, 2 * pr] = 1.0
        esel[64:128, pr, 2 * pr + 1] = 1.0

    in_maps = []
    for c in range(NCORES):
        in_maps.append(dict(
            xT=np.ascontiguousarray(xT_arr[:, :, c * T:(c + 1) * T]),
            WApk=WApk, esel=esel))
    return in_maps


def _prep_expert_weights(W_A, W_B, W_up, W_down):
    key = "w"
    if key in _W_CACHE:
        return _W_CACHE[key]
    # WB: [E,F,R] -> [E, 128(R pad), F]
    WBh = np.zeros((E, 128, F), dtype=BF16)
    WBh[:, :R, :] = W_B.transpose(0, 2, 1).astype(BF16)
    # WUP: [E,F,H] -> ft-major [E, 128, 16, 8, 128]
    #   [e, h%128, ft, h//128, f%128], f = ft*128 + (f%128)
    WUPh = np.ascontiguousarray(
        W_up.transpose(0, 2, 1).reshape(E, 8, 128, 16, 128)
        .transpose(0, 2, 3, 1, 4)).astype(BF16)
    # WDN: [E,H,F] -> [E, 128, 16, H]  (f = k*128 + p)
    WDNh = np.ascontiguousarray(
        W_down.transpose(0, 2, 1).reshape(E, 16, 128, H).transpose(0, 2, 1, 3)
    ).astype(BF16)
    _W_CACHE[key] = (WBh, WUPh, WDNh)
    return _W_CACHE[key]


def kernel(hidden_states, W_A, W_B, W_up, W_down):
    global LAST_RESULT
    trace = _maybe_install_trace_hook()
    from concourse import bass_utils

    f32 = np.float32
    x2d = np.ascontiguousarray(
        np.asarray(hidden_states, dtype=f32).reshape(N, H))
    W_A = np.asarray(W_A, dtype=f32)
    W_B = np.asarray(W_B, dtype=f32)
    W_up = np.asarray(W_up, dtype=f32)
    W_down = np.asarray(W_down, dtype=f32)

    # ---- launch 1: gate ----
    nc1 = _get_nc("gate_" + GATE_DT, _build_nc_gate, GATE_DT)
    in1 = _prep_gate_inputs(x2d, W_A)
    res1 = bass_utils.run_bass_kernel_spmd(
        nc1, in1, core_ids=list(range(NCORES)), trace=trace)

    # w_full[token, expert] (device-computed dense combine weights) and
    # gh (device fp32 gate activations, shipped as bf16) per expert
    w_full = np.empty((N, E), dtype=f32)
    ghG = np.empty((E, R, N), dtype=BF16)
    for c in range(NCORES):
        arr = res1.results[c]["w_all"]                      # [128, 4, E]
        w_full[c * T:(c + 1) * T] = arr.transpose(1, 0, 2).reshape(T, E)
        ghc = res1.results[c]["gh_bf"]                      # [128, 4, T]
        for pr in range(4):
            for half in range(2):
                ghG[2 * pr + half, :, c * T:(c + 1) * T] = \
                    ghc[half * 64:(half + 1) * 64, pr, :]

    # ---- host: group tokens by expert (all-to-all dispatch) ----
    tok_lists = []
    counts = []
    for e in range(E):
        tl = np.nonzero(w_full[:, e] > 0.0)[0]
        tok_lists.append(tl)
        counts.append(len(tl))
    maxc = max(counts)
    K2 = ((maxc + 23 + 127) // 128) * 128                   # capacity w/ pad

    WBh, WUPh, WDNh = _prep_expert_weights(W_A, W_B, W_up, W_down)
    x_bf = x2d.astype(BF16)

    nc2 = _get_nc(("ffn", K2), _build_nc_ffn, K2)
    in2 = []
    for e in range(E):
        tl = tok_lists[e]
        cnt = counts[e]
        xg = np.zeros((H, K2), dtype=BF16)
        xg[:, :cnt] = x_bf[tl].T                            # [H, cnt]
        XG = np.ascontiguousarray(
            xg.reshape(8, 128, K2).transpose(1, 0, 2))      # [128, 8, K2]
        GHG = np.zeros((128, K2), dtype=BF16)
        GHG[:R, :cnt] = ghG[e][:, tl]
        wbc = np.zeros((128, K2), dtype=f32)
        wbc[:, :cnt] = w_full[tl, e][None, :]
        in2.append(dict(XG=XG, GHG=GHG, WB=WBh[e], WUP=WUPh[e],
                        WDN=WDNh[e], WBC=wbc))
    res2 = bass_utils.run_bass_kernel_spmd(
        nc2, in2, core_ids=list(range(NCORES)), trace=trace)

    # ---- host unshard: scatter-add the weighted expert partials ----
    out = np.zeros((N, H), dtype=f32)
    for e in range(E):
        cnt = counts[e]
        Y = res2.results[e]["Y"]                            # [128, 8, K2] bf16
        y = Y.transpose(2, 1, 0).reshape(K2, H)[:cnt].astype(f32)
        out[tok_lists[e]] += y

    t1 = res1.exec_time_ns
    t2 = res2.exec_time_ns
    total = (t1 or 0) + (t2 or 0) if (t1 is not None or t2 is not None) else None
    tr = res2.instructions_and_trace or res1.instructions_and_trace
    LAST_RESULT = _Result(total if (t1 or t2) else None, tr, [t1, t2])

    return out.reshape(B, S, H)
